# revision 34
# baseline (speedup 1.0000x reference)
"""CapsNet-CIFAR100 forward pass on 8 Trainium2 NeuronCores.

Data-parallel over batch (8 images/core); conv stem + primary caps as
matmuls; dynamic routing reformulated so every 26M-element u_hat pass is
either produced or consumed by the TensorEngine:
  pass 0: s0 = sum_i u_hat directly via dense-u matmuls (u_hat never formed)
  pass 1/2: u_hat chunks via block-diag-u matmuls -> PSUM; ACT exits to
  bf16 SBUF; logit path (dm/dh) on Pool, reduce/softmax-scale/p16 on DVE,
  exp on ACT; i-sums back on PE.

All heavy operands (x, conv weights, routing weight wr) are bf16 in DRAM
and in the PE: halves the 105MB-per-pass wr HBM stream and runs the PE at
1 cycle/row instead of fp32's 4. Accumulation stays fp32 in PSUM; the
softmax/c-coefficient path keeps the (c - 0.01) + 0.01*s0 decomposition so
bf16 noise only scales with the routing *corrections*, not the mean term.

Capsule chunking: chunk cb in 0..127, H=cb//64, r=cb%64; the chunk's 16
capsules are co in {128H+64cp+r : cp in 0,1} x oh in 0..7, dim k=ow.
Partition index within chunk: p = cp*64 + oh*8 + ow.
conv2 runs "transposed" (output partitions = (b%2, oh, ow), free = co) so
the u -> U_BD chunk gather is 32 contiguous [64,64] SBUF DMAs.
"""

from contextlib import ExitStack

import numpy as np
import ml_dtypes
import concourse.bass as bass
import concourse.mybir as mybir
import concourse.tile as tile
from concourse import bacc
from concourse import bass_utils

F32 = mybir.dt.float32
BF16 = mybir.dt.bfloat16
AF = mybir.ActivationFunctionType
ALU = mybir.AluOpType
AX = mybir.AxisListType

N_CORES = 8
B = 8            # batch per core
EPS = 1e-8

_CACHE = {}


def _build():
    nc = bacc.Bacc("TRN2", target_bir_lowering=False, debug=False,
                   num_devices=N_CORES)

    xd = nc.dram_tensor("x_im", [3, 81, B, 24, 24], BF16, kind="ExternalInput").ap()
    w1d = nc.dram_tensor("w1t", [3, 81, 256], BF16, kind="ExternalInput").ap()
    cbd = nc.dram_tensor("cb", [256, 1], F32, kind="ExternalInput").ap()
    w2d = nc.dram_tensor("w2t", [2, 128, 81, 256], BF16, kind="ExternalInput").ap()
    pbd = nc.dram_tensor("pb", [1, 256], F32, kind="ExternalInput").ap()
    wrd = nc.dram_tensor("wr", [128, 128, 1600], BF16, kind="ExternalInput").ap()
    mkd = nc.dram_tensor("mask", [128, 128], BF16, kind="ExternalInput").ap()
    seld = nc.dram_tensor("sel", [128, 8], BF16, kind="ExternalInput").ap()
    gd = nc.dram_tensor("gmat", [128, 16], F32, kind="ExternalInput").ap()
    grd = nc.dram_tensor("grepT", [16, 128], F32, kind="ExternalInput").ap()
    brd = nc.dram_tensor("brepT", [8, 128], BF16, kind="ExternalInput").ap()
    vout = nc.dram_tensor("v_out", [B, 100, 16], F32, kind="ExternalOutput").ap()

    # od-tile split: (psum tile key, offset within tile, od range)
    QS = [("uhA", 0, 0, 512), ("uhA", 512, 512, 1024),
          ("uhB", 0, 1024, 1536), ("uhB", 512, 1536, 1600)]

    with tile.TileContext(nc) as tc:
        with ExitStack() as stack:
            cpool = stack.enter_context(tc.tile_pool(name="consts", bufs=1))
            apool = stack.enter_context(tc.tile_pool(name="acts", bufs=1))
            wpool = stack.enter_context(tc.tile_pool(name="work", bufs=3))
            vpool = stack.enter_context(tc.tile_pool(name="vsmall", bufs=1))

            # ---------- stage A: conv1 [B,3,32,32] -> h [256, B, 24, 24] ----------
            w1sb = cpool.tile([81, 3, 256], BF16, name="w1sb")
            nc.sync.dma_start(out=w1sb, in_=w1d.rearrange("c k o -> k c o"))
            cbsb = cpool.tile([128, 2, 1], F32, name="cbsb")
            nc.sync.dma_start(out=cbsb, in_=cbd.rearrange("(t p) one -> p t one", p=128))
            pbrep = cpool.tile([128, 256], F32, name="pbrep")
            nc.sync.dma_start(
                out=pbrep,
                in_=bass.AP(tensor=pbd.tensor, offset=0, ap=[[0, 128], [1, 256]]))
            epssb = cpool.tile([128, 1], F32, name="epssb")
            nc.vector.memset(epssb, EPS)
            gsb = cpool.tile([128, 16], F32, name="gsb")
            nc.sync.dma_start(out=gsb, in_=gd)
            grsb = cpool.tile([16, 128], F32, name="grsb")
            nc.sync.dma_start(out=grsb, in_=grd)
            brsb = cpool.tile([8, 128], BF16, name="brsb")
            nc.sync.dma_start(out=brsb, in_=brd)

            hctx = tc.tile_pool(name="hp", bufs=1)
            hpool = hctx.__enter__()
            hsb = [hpool.tile([128, B, 24, 24], BF16, name="hsb", tag=f"h{c}") for c in range(2)]
            with tc.tile_pool(name="imp", bufs=1) as impool, \
                 tc.tile_pool(name="psc", bufs=2, space="PSUM") as pscpool:
                im = [impool.tile([81, B, 24, 24], BF16, name="im", tag=f"im{ci}") for ci in range(3)]
                for ci in range(3):
                    # im2col done host-side; one contiguous DMA per channel
                    nc.sync.dma_start(out=im[ci], in_=xd[ci])

                for oc in range(2):
                    for ns in range(9):
                        ph = pscpool.tile([128, 512], F32, name="ph", tag="pconv")
                        for ci in range(3):
                            nc.tensor.matmul(
                                ph,
                                lhsT=w1sb[:, ci, oc * 128:(oc + 1) * 128],
                                rhs=im[ci].rearrange("k b h w -> k (b h w)")[:, ns * 512:(ns + 1) * 512],
                                start=(ci == 0), stop=(ci == 2),
                            )
                        nc.scalar.activation(
                            hsb[oc].rearrange("p b h w -> p (b h w)")[:, ns * 512:(ns + 1) * 512],
                            ph, AF.Relu, bias=cbsb[:, oc],
                        )

            # ---------- stage B+C: conv2 (transposed) + squash -> u_B[bp] ----------
            # conv2-B: psum [(b%2, oh, ow)=128, co=256] per b-pair bp
            # lhsT = h-shifted slice [ci, (2b, oh, ow)]; rhs = w2 [ci, co]
            ub = [apool.tile([128, 256], F32, name="ub", tag=f"ub{bp}") for bp in range(4)]
            w2ctx = tc.tile_pool(name="w2", bufs=4)
            w2pool = w2ctx.__enter__()
            psc2ctx = tc.tile_pool(name="psc2", bufs=1, space="PSUM")
            psc2pool = psc2ctx.__enter__()
            p2sb = [apool.tile([128, 256], F32, name="p2sb", tag=f"p2sb{bp}") for bp in range(4)]
            p2ps = [psc2pool.tile([128, 256], F32, name="p2ps", tag=f"p2ps{bp}")
                    for bp in range(4)]
            nmm = [0, 0, 0, 0]
            for g in range(9):
                w2g = [w2pool.tile([128, 9, 256], BF16, name="w2g", tag="w2g") for _ in range(2)]
                for cic in range(2):
                    nc.sync.dma_start(out=w2g[cic], in_=w2d[cic, :, g * 9:(g + 1) * 9, :])
                for j in range(9):
                    khw = g * 9 + j
                    kh, kw = khw // 9, khw % 9
                    for cic in range(2):
                        hshift = wpool.tile([128, B, 8, 8], BF16, name="hshift", tag="hshift")
                        if cic == 0:
                            nc.vector.tensor_copy(
                                hshift, hsb[cic][:, :, kh:kh + 16:2, kw:kw + 16:2])
                        else:
                            nc.scalar.copy(
                                hshift, hsb[cic][:, :, kh:kh + 16:2, kw:kw + 16:2])
                        hflat = hshift.rearrange("p b h w -> p (b h w)")
                        for bp in range(4):
                            nc.tensor.matmul(
                                p2ps[bp],
                                lhsT=hflat[:, bp * 128:(bp + 1) * 128],
                                rhs=w2g[cic][:, j, :],
                                start=(nmm[bp] == 0), stop=(nmm[bp] == 161),
                            )
                            nmm[bp] += 1
            for bp in range(4):
                # exit psum + bias (pcap_b broadcast along partitions)
                nc.vector.tensor_tensor(out=p2sb[bp], in0=p2ps[bp], in1=pbrep,
                                        op=ALU.add)
            w2ctx.__exit__(None, None, None)
            psc2ctx.__exit__(None, None, None)
            hctx.__exit__(None, None, None)   # h dead after conv2: free 18KB/part

            # squash over ow (= partition subgroups of 8) via G-matmul
            with tc.tile_pool(name="psn", bufs=2, space="PSUM") as psnpool:
                for bp in range(4):
                    sq = wpool.tile([128, 256], F32, name="sq", tag="sq")
                    nc.vector.tensor_mul(sq, p2sb[bp], p2sb[bp])
                    n2ps = psnpool.tile([16, 256], F32, name="n2ps", tag="n2ps")
                    nc.tensor.matmul(n2ps, lhsT=gsb, rhs=sq, start=True, stop=True)
                    # f = n2/(1+n2) * rsqrt(n2+eps)  on [16, 256]
                    n2 = wpool.tile([16, 256], F32, name="n2", tag="n2")
                    nc.scalar.activation(n2, n2ps, AF.Copy)
                    r1 = wpool.tile([16, 256], F32, name="r1", tag="r1")
                    nc.vector.tensor_scalar_add(r1, in0=n2, scalar1=1.0)
                    nc.vector.reciprocal(r1, r1)
                    q = wpool.tile([16, 256], F32, name="q", tag="q")
                    nc.scalar.activation(q, n2, AF.Sqrt, bias=epssb[:16])
                    nc.vector.reciprocal(q, q)
                    f = wpool.tile([16, 256], F32, name="f", tag="f")
                    nc.vector.tensor_mul(f, n2, r1)
                    nc.vector.tensor_mul(f, f, q)
                    # replicate f over ow (partition groups of 8) on the PE
                    frps = psnpool.tile([128, 256], F32, name="frps", tag="frps")
                    nc.tensor.matmul(frps, lhsT=grsb, rhs=f, start=True, stop=True)
                    nc.vector.tensor_mul(ub[bp], p2sb[bp], frps)

            # ---------- stage D: U_BD[H] [128=(cp,s), 8 b, 64 r] ----------
            rpool = stack.enter_context(tc.tile_pool(name="rconsts", bufs=1))
            ubd = [rpool.tile([128, B, 64], F32, name="ubd", tag=f"ubd{H}") for H in range(2)]
            for H in range(2):
                for cp in range(2):
                    for b in range(B):
                        bp, bl = b // 2, b % 2
                        nc.sync.dma_start(
                            out=ubd[H][cp * 64:(cp + 1) * 64, b, :],
                            in_=ub[bp][bl * 64:(bl + 1) * 64,
                                       128 * H + 64 * cp:128 * H + 64 * cp + 64],
                        )

            ubd2 = [rpool.tile([128, 64, B], BF16, name="ubd2", tag=f"ubd2{H}")
                    for H in range(2)]
            for H in range(2):
                nc.vector.tensor_copy(
                    ubd2[H],
                    bass.AP(tensor=ubd[H].tensor, offset=ubd[H].offset,
                            ap=[list(ubd[H].ap[0]), [1, 64], [64, B]]))

            masksb = rpool.tile([128, 16, 8], BF16, name="masksb")
            nc.sync.dma_start(out=masksb, in_=mkd.rearrange("p (i b) -> p i b", b=8))
            selsb = rpool.tile([128, 8], BF16, name="sel16")
            nc.sync.dma_start(out=selsb, in_=seld)

            # routing tensors use (d, o) free-dim order (d outer, o inner)
            s0keep = rpool.tile([8, 16, 100], F32, name="s0keep")
            vreps = [rpool.tile([128, 16, 100], BF16, name="vrep", tag=f"vrep{i}")
                     for i in range(2)]
            vkeep = rpool.tile([8, 16, 100], F32, name="vkeep")
            v2sb = rpool.tile([8, 100, 16], F32, name="v2sb")
            # block-diag u for every chunk, built once during pass 0
            bdall = [rpool.tile([128, 64, 16, 8], BF16, name="bdall", tag=f"bdall{H}")
                     for H in range(2)]

            wrpool = stack.enter_context(tc.tile_pool(name="wrp", bufs=5))
            dpool = stack.enter_context(tc.tile_pool(name="deep", bufs=4))
            psuhpool = stack.enter_context(tc.tile_pool(name="psuh", bufs=1, space="PSUM"))
            psspool = stack.enter_context(tc.tile_pool(name="pss", bufs=1, space="PSUM"))

            def stream_wr(cb):
                t = wrpool.tile([128, 1600], BF16, name="wrt", tag="wrt")
                nc.sync.dma_start(out=t, in_=wrd[cb])
                return t

            def uh_psum_pair():
                uhA = psuhpool.tile([128, 1024], F32, name="uhA", tag="uhA")
                uhB = psuhpool.tile([128, 576], F32, name="uhB", tag="uhB")
                return {"uhA": uhA, "uhB": uhB}

            def squash_psum(S, scale, out16, outf32=None, base=None):
                """v = squash(S*scale + 0.01*base): S psum [8, 2048(:1600)], (d,o)."""
                Sc = vpool.tile([8, 16, 100], F32, name="vsc", tag="vsc")
                if base is None:
                    nc.scalar.activation(Sc.rearrange("p d o -> p (d o)"), S[:, :1600], AF.Copy)
                else:
                    nc.vector.scalar_tensor_tensor(
                        out=Sc.rearrange("p d o -> p (d o)"),
                        in0=base.rearrange("p d o -> p (d o)"), scalar=0.01,
                        in1=S[:, :1600], op0=ALU.mult, op1=ALU.add)
                Sv = Sc
                sq = vpool.tile([8, 16, 100], F32, name="vsq", tag="vtmp")
                nc.vector.tensor_mul(sq, Sv, Sv)
                # n2[b, o] = sum_d sq: fold chain over the outer d axis
                n8 = vpool.tile([8, 8, 100], F32, name="vn8", tag="vn8")
                nc.vector.tensor_tensor(out=n8, in0=sq[:, 0:8], in1=sq[:, 8:16], op=ALU.add)
                n4 = vpool.tile([8, 4, 100], F32, name="vn4", tag="vn4")
                nc.vector.tensor_tensor(out=n4, in0=n8[:, 0:4], in1=n8[:, 4:8], op=ALU.add)
                n2b = vpool.tile([8, 2, 100], F32, name="vn2b", tag="vn2b")
                nc.vector.tensor_tensor(out=n2b, in0=n4[:, 0:2], in1=n4[:, 2:4], op=ALU.add)
                n2 = vpool.tile([8, 100], F32, name="vn2", tag="vn2")
                nc.vector.tensor_tensor(out=n2, in0=n2b[:, 0], in1=n2b[:, 1], op=ALU.add)
                if scale != 1.0:
                    nc.vector.tensor_scalar_mul(n2, in0=n2, scalar1=scale * scale)
                r1 = vpool.tile([8, 100], F32, name="vr1", tag="vr1")
                nc.vector.tensor_scalar_add(r1, in0=n2, scalar1=1.0)
                nc.vector.reciprocal(r1, r1)
                q = vpool.tile([8, 100], F32, name="vq", tag="vq")
                nc.scalar.activation(q, n2, AF.Sqrt, bias=epssb[:8])
                nc.vector.reciprocal(q, q)
                f = vpool.tile([8, 100], F32, name="vf", tag="vf")
                nc.vector.tensor_mul(f, n2, r1)
                nc.vector.tensor_mul(f, f, q)
                if scale != 1.0:
                    nc.vector.tensor_scalar_mul(f, in0=f, scalar1=scale)
                tgt = outf32 if outf32 is not None else vpool.tile(
                    [8, 16, 100], F32, name="vtmp", tag="vtmp")
                nc.vector.tensor_tensor(out=tgt, in0=Sv,
                                        in1=f.unsqueeze(1).broadcast_to([8, 16, 100]),
                                        op=ALU.mult)
                nc.vector.tensor_copy(out16, tgt)

            def fill_vrep(v16, vrep):
                # vrep[p, :] = v16[p % 8, :] via PE broadcast (lhsT = brsb)
                ps = uh_psum_pair()
                vf = vrep.rearrange("p d o -> p (d o)")
                for (key, po, n0, n1) in QS:
                    nc.tensor.matmul(ps[key][:, po:po + n1 - n0], lhsT=brsb,
                                     rhs=v16[:, n0:n1], start=True, stop=True)
                nc.scalar.activation(vf[:, 0:1024], ps["uhA"], AF.Copy)
                nc.scalar.activation(vf[:, 1024:1600], ps["uhB"], AF.Copy)

            # ---------- pass 0 ----------
            s0ps = psspool.tile([8, 2048], F32, name="s0ps", tag="spsum")
            for cb in range(128):
                H, r = cb // 64, cb % 64
                wrt = stream_wr(cb)
                for qi, (key, po, n0, n1) in enumerate(QS):
                    nc.tensor.matmul(s0ps[:, n0:n1],
                                     lhsT=ubd2[H][:, r, :],
                                     rhs=wrt[:, n0:n1],
                                     start=(cb == 0), stop=(cb == 127))
            # build bdall during pass 0 (DVE is idle there)
            for H in range(2):
                nc.vector.tensor_tensor(
                    out=bdall[H],
                    in0=bass.AP(tensor=ubd2[H].tensor, offset=ubd2[H].offset,
                                ap=[list(ubd2[H].ap[0]), [8, 64], [0, 16], [1, 8]]),
                    in1=bass.AP(tensor=masksb.tensor, offset=masksb.offset,
                                ap=[list(masksb.ap[0]), [0, 64], [8, 16], [1, 8]]),
                    op=ALU.mult)
            v16 = vpool.tile([8, 16, 100], BF16, name="v16")
            nc.scalar.activation(s0keep.rearrange("p d o -> p (d o)"),
                                 s0ps[:, :1600], AF.Copy)
            squash_psum(s0ps, 0.01, v16, outf32=vkeep)
            fill_vrep(v16.rearrange("p d o -> p (d o)"), vreps[1])

            # ---------- passes 1, 2 ----------
            for t in (1, 2):
                sps = psspool.tile([8, 2048], F32, name="sps", tag="spsum")
                vrep = vreps[t % 2]
                for cb in range(128):
                    H, r = cb // 64, cb % 64
                    wrt = stream_wr(cb)
                    bdv = bass.AP(
                        tensor=bdall[H].tensor, offset=bdall[H].offset + r * 128,
                        ap=[list(bdall[H].ap[0]), [1, 128]])
                    ps = uh_psum_pair()
                    uh16 = dpool.tile([128, 16, 100], BF16, name="uh16", tag="uh16")
                    uhf = uh16.rearrange("p d o -> p (d o)")
                    for (key, po, n0, n1) in QS:
                        nc.tensor.matmul(ps[key][:, po:po + n1 - n0],
                                         lhsT=bdv, rhs=wrt[:, n0:n1],
                                         start=True, stop=True)
                    nc.scalar.activation(uhf[:, 0:1024], ps["uhA"], AF.Copy)
                    nc.scalar.activation(uhf[:, 1024:1600], ps["uhB"], AF.Copy)
                    # logit dot: pass 1 uses v0, pass 2 uses v0+v1
                    # (b2 = b1 + v1.u_hat = (v0+v1).u_hat exactly)
                    # all-2x fold chain over the outer d axis
                    dm = dpool.tile([128, 16, 100], BF16, name="dm", tag="dm")
                    nc.vector.tensor_mul(dm, uh16, vrep)
                    dh = dpool.tile([128, 8, 100], BF16, name="dh", tag="dh")
                    nc.vector.tensor_tensor(out=dh, in0=dm[:, 0:8],
                                            in1=dm[:, 8:16], op=ALU.add)
                    dh4 = dpool.tile([128, 4, 100], BF16, name="dh4", tag="dh4")
                    nc.vector.tensor_tensor(out=dh4, in0=dh[:, 0:4],
                                            in1=dh[:, 4:8], op=ALU.add)
                    dh2 = dpool.tile([128, 2, 100], BF16, name="dh2", tag="dh2")
                    nc.vector.tensor_tensor(out=dh2, in0=dh4[:, 0:2],
                                            in1=dh4[:, 2:4], op=ALU.add)
                    logit = wpool.tile([128, 100], BF16, name="logit", tag="logit")
                    with nc.allow_low_precision(reason="logits tiny; bf16 ok"):
                        nc.vector.tensor_tensor(out=logit, in0=dh2[:, 0],
                                                in1=dh2[:, 1], op=ALU.add)
                    e = wpool.tile([128, 100], F32, name="e", tag="e")
                    z = wpool.tile([128, 1], F32, name="z", tag="z")
                    nc.scalar.activation(e, logit, AF.Exp, accum_out=z)
                    nc.vector.reciprocal(z, z)
                    c16 = wpool.tile([128, 100], BF16, name="c16", tag="c16")
                    nc.scalar.activation(c16, e, AF.Copy, scale=z, bias=-0.01)
                    p16 = dpool.tile([128, 16, 100], BF16, name="p16", tag="p16")
                    nc.vector.tensor_tensor(
                        out=p16, in0=uh16,
                        in1=c16.unsqueeze(1).broadcast_to([128, 16, 100]),
                        op=ALU.mult)
                    if cb % 2 == 0:
                        p16keep = p16
                    else:
                        # pair-sum on DVE halves the sps matmul row count (PE
                        # is pstate-limited; trade 0.45us DVE for 0.85us PE)
                        pp = wpool.tile([128, 16, 100], BF16, name="pp", tag="pp")
                        nc.vector.tensor_tensor(out=pp, in0=p16keep, in1=p16,
                                                op=ALU.add)
                        ppf = pp.rearrange("p d o -> p (d o)")
                        for (key, po, n0, n1) in QS:
                            nc.tensor.matmul(sps[:, n0:n1], lhsT=selsb,
                                             rhs=ppf[:, n0:n1],
                                             start=(cb == 1), stop=(cb == 127))
                if t == 1:
                    v1f = vpool.tile([8, 16, 100], F32, name="v1f", tag="v1f")
                    squash_psum(sps, 1.0, v16, outf32=v1f, base=s0keep)
                    # pass-2 logits need (v0 + v1) . u_hat
                    vsum = vpool.tile([8, 16, 100], F32, name="vsum", tag="vsum")
                    nc.vector.tensor_tensor(out=vsum, in0=v1f, in1=vkeep, op=ALU.add)
                    nc.vector.tensor_copy(v16, vsum)
                    fill_vrep(v16.rearrange("p d o -> p (d o)"), vreps[0])
                else:
                    vfin = vpool.tile([8, 16, 100], F32, name="vfin", tag="v1f")
                    squash_psum(sps, 1.0, v16, outf32=vfin, base=s0keep)
                    # transpose (d,o) -> (o,d) for the output
                    nc.vector.tensor_copy(
                        v2sb,
                        bass.AP(tensor=vfin.tensor, offset=vfin.offset,
                                ap=[list(vfin.ap[0]), [1, 100], [100, 16]]))
                    nc.sync.dma_start(out=vout, in_=v2sb)

    nc.compile()
    return nc


def _host_prep(x, conv_w, conv_b, pcap_w, pcap_b, W):
    BF = ml_dtypes.bfloat16
    x = np.ascontiguousarray(np.asarray(x, np.float32))
    conv_w = np.asarray(conv_w, np.float32)
    conv_b = np.asarray(conv_b, np.float32)
    pcap_w = np.asarray(pcap_w, np.float32)
    pcap_b = np.asarray(pcap_b, np.float32)
    W = np.asarray(W, np.float32)

    w1t = np.ascontiguousarray(conv_w.reshape(256, 3, 81).transpose(1, 2, 0)).astype(BF)
    cb = np.ascontiguousarray(conv_b.reshape(256, 1))
    w2t = np.ascontiguousarray(
        pcap_w.transpose(1, 2, 3, 0).reshape(2, 128, 81, 256)).astype(BF)
    pb = np.ascontiguousarray(pcap_b.reshape(1, 256))
    # wr[cb=(H,r)][p=(cp,oh,ow)][(d,o)] = W[o, (128H+64cp+r)*8+oh, d, ow]
    # (d outer, o inner: keeps broadcast/fold APs step-1 for DVE 2x mode)
    arr = W.transpose(1, 3, 0, 2)                # [i=2048, k=8, o=100, d=16]
    arr = arr.reshape(2, 2, 64, 8, 8, 100, 16)   # [H, cp, r, oh, k, o, d]
    arr = arr.transpose(0, 2, 1, 3, 4, 6, 5)     # [H, r, cp, oh, k, d, o]
    wr = np.ascontiguousarray(arr.reshape(128, 128, 1600)).astype(BF)

    mask = np.zeros((128, 128), np.float32)
    for p in range(128):
        mask[p, (p // 8) * 8:(p // 8) * 8 + 8] = 1.0
    mask = mask.astype(BF)
    sel = np.zeros((128, 8), np.float32)
    for p in range(128):
        sel[p, p % 8] = 1.0
    sel = sel.astype(BF)
    g = np.zeros((128, 16), np.float32)
    for p in range(128):
        g[p, p // 8] = 1.0
    grepT = np.ascontiguousarray(g.T)            # [16, 128] f32
    brepT = np.zeros((8, 128), np.float32)
    for p in range(128):
        brepT[p % 8, p] = 1.0
    brepT = brepT.astype(BF)

    shared = {"w1t": w1t, "cb": cb, "w2t": w2t, "pb": pb, "wr": wr,
              "mask": mask, "sel": sel, "gmat": g, "grepT": grepT,
              "brepT": brepT}
    # host-side im2col: x_im[ci, kh*9+kw, b, oh, ow] = x[b, ci, kh+oh, kw+ow]
    xim = np.empty((64, 3, 81, 24, 24), np.float32)
    for kh in range(9):
        for kw in range(9):
            xim[:, :, kh * 9 + kw] = x[:, :, kh:kh + 24, kw:kw + 24]
    xim = xim.astype(BF)

    in_maps = []
    for c in range(N_CORES):
        m = dict(shared)
        m["x_im"] = np.ascontiguousarray(
            xim[c * B:(c + 1) * B].transpose(1, 2, 0, 3, 4))
        in_maps.append(m)
    return in_maps


def run(inputs, trace=False, **kw):
    if "nc" not in _CACHE:
        _CACHE["nc"] = _build()
    nc = _CACHE["nc"]
    in_maps = _host_prep(**inputs)
    res = bass_utils.run_bass_kernel_spmd(
        nc, in_maps, core_ids=list(range(N_CORES)), trace=trace, **kw)
    return res


def kernel(**inputs):
    res = run(inputs)
    v = np.concatenate([res.results[i]["v_out"] for i in range(N_CORES)], axis=0)
    return v
